# revision 10
# baseline (speedup 1.0000x reference)
"""Dice + contrastive loss on 8 Trainium2 NeuronCores.

Sharding: every input tensor [16,1,512,512] is flattened to [16, 262144]
and sharded along the *pixel* axis (32768 pixels per image per core).
Every term of the loss becomes a local partial reduction:

  - dice:   sum(sigmoid(pred)), sum(sigmoid(pred)*gt), sum(gt)   (scalars)
  - pos:    sum((mask*(s1-s2))^2) per image               (diag of a Gram)
  - sq1/2:  sum(s1^2), sum(s2^2) per image                (diag of a Gram)
  - cross:  s1 @ s2.T (16x16 Gram), contraction over pixels

Schedule (per core), honed against perfetto traces:
  - ~7us fixed engine boot; first DMA trigger ~7.1us; each DMA->compute
    edge pays ~2us completion-receipt latency, so transfer order matters.
  - ACT spine: dummy sigmoid preloads the spline table under the boot,
    pred halves FIRST (their accum_out gives sum(p) and they overlap the
    DMA fill), then one instr per (in1_g|in2_g) pair written straight
    into the Gram-pack layout.
  - sum(gt) and sum(p*gt) are ones-stationary matmuls on the otherwise
    idle early PE (psE/psF, one LDW), evacuated + DMA'd mid-kernel.
  - DVE does only 2x-mode work: prod = p*gt, d = s1-s2, dm = d*mask.
  - PE warm-up matmuls flip the HAM clock gate to 2.4 GHz before the 3
    PSUM-accumulated Grams (psA = s1.[s1|s2], psB = s2.s2, psC = dm.dm)
    which then track the ACT pair cadence (~1.7us/pair warm).
  - tiny cross-core combine (a few KiB per core) happens on the host.
"""

import os
import sys

sys.path.insert(0, "/opt/trn_rl_repo")

import numpy as np
import ml_dtypes

import concourse.bass as bass
import concourse.tile as tile
from concourse import bacc, mybir
from concourse.bass_utils import run_bass_kernel_spmd

TAU = 0.1
DICE_SMOOTH = 0.1
WEIGHT = 1.0

NCORES = 8
B = 16                      # batch (images)
NPIX = 512 * 512            # pixels per image
PIX = NPIX // NCORES        # pixels per image per core = 32768
P = 128                     # partitions
F = PIX // P                # free columns per image per core = 256
T = 32                      # Gram contraction chunks (each covers 8 f-columns)
S = F // T                  # sub-columns per chunk = 8
G = 4                       # pair groups
TG = T // G                 # t-chunks per pair group = 8
BF = B * F                  # 4096

F32 = mybir.dt.float32
BF16 = mybir.dt.bfloat16
FP8 = mybir.dt.float8e4
NP_BF16 = ml_dtypes.bfloat16
NP_FP8 = ml_dtypes.float8_e4m3
AF = mybir.ActivationFunctionType
ALU = mybir.AluOpType

N_WARM = int(os.environ.get("N_WARM", "12"))


def _build_program():
    nc = bacc.Bacc("TRN2", target_bir_lowering=False, debug=False,
                   num_devices=NCORES)

    # x8 (fp8): [pair0 | pair1 | pair2 | pair3 | pred]
    # x16 (bf16): [mask | gt]
    d_x8 = nc.dram_tensor("x8", [P, 4 * 2048 + BF], FP8, kind="ExternalInput")
    d_x16 = nc.dram_tensor("x16", [P, 2 * BF], BF16, kind="ExternalInput")

    o_grams = nc.dram_tensor("grams", [P, 4 * P + 8], F32, kind="ExternalOutput")
    o_small = nc.dram_tensor("small", [1, 1024], F32, kind="ExternalOutput")

    with tile.TileContext(nc) as tc:
        with tc.tile_pool(name="main", bufs=1) as pool:
            t_pair = [pool.tile([P, 2048], FP8, name=f"t_pair{g}", tag=f"t_pair{g}")
                      for g in range(G)]
            t_mask = pool.tile([P, BF], BF16, tag="t_mask")
            t_gt = pool.tile([P, BF], BF16, tag="t_gt")
            t_pred = pool.tile([P, BF], FP8, tag="t_pred")
            # Gram-pack layout, col = t*256 + h*128 + (s*16+b); h=0: s1, h=1: s2
            s12 = pool.tile([P, 2 * BF], BF16, tag="s12")
            dd = pool.tile([P, 2 * BF], BF16, tag="dd")   # h=0: d, h=1: dm
            t_p = pool.tile([P, BF], BF16, tag="t_p")     # sigmoid(pred), natural
            prod = pool.tile([P, BF], BF16, tag="prod")   # p * gt
            grams_sb = pool.tile([P, 4 * P + 8], F32, tag="grams_sb")
            small_sb = pool.tile([P, 1024], F32, tag="small_sb")
            ones = pool.tile([P, 1], BF16, tag="ones")
            warm = pool.tile([P, 512], BF16, tag="warm")
            dum = pool.tile([P, 8], BF16, tag="dum")
            stats = grams_sb  # sum(p) accum cols 512..513

            with tc.tile_pool(name="psum", bufs=1, space="PSUM") as psum_pool:
                psA = psum_pool.tile([P, 2 * P], F32, tag="psA")
                psB = psum_pool.tile([P, P], F32, tag="psB")
                psC = psum_pool.tile([P, P], F32, tag="psC")
                psW = psum_pool.tile([P, 512], F32, tag="psW")
                psE = psum_pool.tile([1, 512], F32, tag="psE")  # sum(gt) partials
                psF = psum_pool.tile([1, 512], F32, tag="psF")  # sum(p*gt) partials

                x8 = d_x8.ap()
                x16 = d_x16.ap()
                PR0 = 4 * 2048            # pred offset in x8
                GT0 = BF                  # gt offset in x16

                # ---- input DMAs (emission order = priority) ----
                nc.sync.dma_start(t_pred[:, :2048], x8[:, PR0:PR0 + 2048])
                nc.sync.dma_start(t_pred[:, 2048:], x8[:, PR0 + 2048:PR0 + BF])
                nc.sync.dma_start(t_gt[:, :2048], x16[:, GT0:GT0 + 2048])
                nc.sync.dma_start(t_gt[:, 2048:], x16[:, GT0 + 2048:GT0 + BF])
                nc.sync.dma_start(t_pair[0][:], x8[:, 0:2048])
                nc.sync.dma_start(t_pair[1][:], x8[:, 2048:4096])
                nc.sync.dma_start(t_mask[:, :2048], x16[:, 0:2048])
                nc.sync.dma_start(t_pair[2][:], x8[:, 4096:6144])
                nc.sync.dma_start(t_pair[3][:], x8[:, 6144:8192])
                nc.sync.dma_start(t_mask[:, 2048:], x16[:, 2048:BF])

                # ---- warm-up: ACT table preload + PE HAM unthrottle ----
                nc.vector.memset(dum[:], 0.0)
                nc.vector.memset(warm[:], 0.0)
                nc.vector.memset(ones[:], 1.0)
                nc.scalar.activation(dum[:, 0:1], dum[:, 1:2], AF.Sigmoid)
                for i in range(N_WARM):
                    nc.tensor.matmul(psW[:], warm[:, :P], warm[:],
                                     start=True, stop=True)

                # ---- ACT: pred sigmoid halves first (accum -> sum(p)) ----
                for h in range(2):
                    nc.scalar.activation(t_p[:, h * 2048:(h + 1) * 2048],
                                         t_pred[:, h * 2048:(h + 1) * 2048],
                                         AF.Sigmoid,
                                         accum_out=stats[:, 512 + h:513 + h])
                    # DVE: prod = p * gt (2x mode)
                    nc.vector.tensor_tensor(prod[:, h * 2048:(h + 1) * 2048],
                                            t_p[:, h * 2048:(h + 1) * 2048],
                                            t_gt[:, h * 2048:(h + 1) * 2048],
                                            ALU.mult)

                # ---- PE: sum(gt) / sum(p*gt) via ones-stationary matmuls ----
                for c in range(8):
                    nc.tensor.matmul(psE[:], ones[:], t_gt[:, c * 512:(c + 1) * 512],
                                     start=(c == 0), stop=(c == 7))
                for c in range(8):
                    nc.tensor.matmul(psF[:], ones[:], prod[:, c * 512:(c + 1) * 512],
                                     start=(c == 0), stop=(c == 7))

                # s12/dd chunk views: [p, t, h, c]
                v_s12 = s12[:].rearrange("p (t h c) -> p t h c", h=2, c=P)
                v_dd = dd[:].rearrange("p (t h c) -> p t h c", h=2, c=P)
                v_mask = t_mask[:].rearrange("p (t c) -> p t c", c=P)
                s12r = s12[:]
                ddr = dd[:]

                for g in range(G):
                    # ACT: sigmoid of [in1_g | in2_g] into the pack layout
                    out_v = s12[:, g * 2048:(g + 1) * 2048].rearrange(
                        "p (t h c) -> p h t c", h=2, c=P)
                    nc.scalar.activation(out_v, t_pair[g][:], AF.Sigmoid)

                    ts = slice(g * TG, (g + 1) * TG)
                    # DVE: d = s1 - s2 ; dm = d * mask
                    nc.vector.tensor_tensor(v_dd[:, ts, 0, :],
                                            v_s12[:, ts, 0, :],
                                            v_s12[:, ts, 1, :], ALU.subtract)
                    nc.vector.tensor_tensor(v_dd[:, ts, 1, :],
                                            v_dd[:, ts, 0, :],
                                            v_mask[:, ts, :], ALU.mult)

                    if g == 0:
                        # DVE: evacuate psE/psF mid-kernel; DMA them out
                        nc.vector.tensor_copy(small_sb[0:1, :512], psE[:])
                        nc.vector.tensor_copy(small_sb[0:1, 512:], psF[:])

                    # PE: Grams, PSUM-accumulated across all 32 chunks
                    for t in range(g * TG, (g + 1) * TG):
                        st = dict(start=(t == 0), stop=(t == T - 1))
                        c0, c1, c2 = t * 2 * P, t * 2 * P + P, (t + 1) * 2 * P
                        nc.tensor.matmul(psA[:], s12r[:, c0:c1], s12r[:, c0:c2], **st)
                        nc.tensor.matmul(psB[:], s12r[:, c1:c2], s12r[:, c1:c2], **st)
                        nc.tensor.matmul(psC[:], ddr[:, c1:c2], ddr[:, c1:c2], **st)

                    if g == 0:
                        nc.sync.dma_start(o_small.ap(), small_sb[0:1, :])

                # ---- evacuate PSUM -> SBUF (all on scalar) -> DRAM ----
                nc.scalar.copy(grams_sb[:, :2 * P], psA[:])
                nc.scalar.copy(grams_sb[:, 2 * P:3 * P], psB[:])
                nc.scalar.copy(grams_sb[:, 3 * P:4 * P], psC[:])

                nc.sync.dma_start(o_grams.ap(), grams_sb[:])

    nc.compile()
    return nc


_NC_CACHE = None


def _get_program():
    global _NC_CACHE
    if _NC_CACHE is None:
        _NC_CACHE = _build_program()
    return _NC_CACHE


def _shard_inputs(pred_labeled, gt_labeled, input1, input2, mask):
    flat = {
        "pred": np.asarray(pred_labeled, dtype=np.float32).reshape(B, NPIX),
        "gt": np.asarray(gt_labeled, dtype=np.float32).reshape(B, NPIX),
        "in1": np.asarray(input1, dtype=np.float32).reshape(B, NPIX),
        "in2": np.asarray(input2, dtype=np.float32).reshape(B, NPIX),
        "mask": np.asarray(mask, dtype=np.float32).reshape(B, NPIX),
    }

    def nat(a, sl):   # natural: [P, (b f)]
        return (a[:, sl].reshape(B, P, F).transpose(1, 0, 2)
                .reshape(P, B * F))

    def pack(a, sl):  # Gram pack: [P, (t s b)]
        return (a[:, sl].reshape(B, P, T, S).transpose(1, 2, 3, 0)
                .reshape(P, B * F))

    in_maps = []
    for k in range(NCORES):
        sl = slice(k * PIX, (k + 1) * PIX)
        pk1 = pack(flat["in1"], sl)
        pk2 = pack(flat["in2"], sl)
        x8 = np.empty((P, 4 * 2048 + BF), dtype=np.float32)
        for g in range(G):
            x8[:, g * 2048:g * 2048 + 1024] = pk1[:, g * 1024:(g + 1) * 1024]
            x8[:, g * 2048 + 1024:(g + 1) * 2048] = pk2[:, g * 1024:(g + 1) * 1024]
        x8[:, 4 * 2048:] = nat(flat["pred"], sl)
        x16 = np.empty((P, 2 * BF), dtype=np.float32)
        x16[:, :BF] = pack(flat["mask"], sl)
        x16[:, BF:] = nat(flat["gt"], sl)
        in_maps.append({
            "x8": np.ascontiguousarray(x8).astype(NP_FP8),
            "x16": np.ascontiguousarray(x16).astype(NP_BF16),
        })
    return in_maps


def _block_diag_sum(gmat):
    # [128, 128] with rows (s*16+b1), cols (s*16+b2) -> sum_s of [16,16] blocks
    g = gmat.reshape(S, B, S, B)
    return np.einsum("sbsc->bc", g)


def _combine(results):
    sum_p = sum_pg = sum_g = 0.0
    g1 = np.zeros((B, B), np.float64)
    cr = np.zeros((B, B), np.float64)
    g2 = np.zeros((B, B), np.float64)
    pc = np.zeros((B, B), np.float64)
    for r in results:
        gm = r["grams"].astype(np.float64)
        sm = r["small"].astype(np.float64)
        sum_p += gm[:, 512:514].sum()
        sum_g += sm[0, :512].sum()
        sum_pg += sm[0, 512:].sum()
        g1 += _block_diag_sum(gm[:, :P])
        cr += _block_diag_sum(gm[:, P:2 * P])
        g2 += _block_diag_sum(gm[:, 2 * P:3 * P])
        pc += _block_diag_sum(gm[:, 3 * P:4 * P])

    dice = 1.0 - (2.0 * sum_pg + DICE_SMOOTH) / (sum_p + sum_g + DICE_SMOOTH)

    n = float(NPIX)
    sq1 = np.diag(g1) / n
    sq2 = np.diag(g2) / n
    cross = cr / n
    pos_mse = np.diag(pc) / n

    sim_pos = np.exp(-pos_mse / TAU)
    mse = sq1[:, None] + sq2[None, :] - 2.0 * cross
    sim = np.exp(-mse / TAU)
    sim_neg = (sim * (1.0 - np.eye(B))).sum(axis=1)
    loss_c = float(np.mean(-np.log(sim_pos / (sim_pos + sim_neg))))
    total = dice + WEIGHT * loss_c
    return (np.float32(total), np.float32(dice), 0.0, np.float32(loss_c))


def kernel(pred_labeled, gt_labeled, input1, input2, mask):
    nc = _get_program()
    in_maps = _shard_inputs(pred_labeled, gt_labeled, input1, input2, mask)
    res = run_bass_kernel_spmd(nc, in_maps, core_ids=list(range(NCORES)),
                               trace=bool(int(os.environ.get("KERNEL_TRACE", "0"))))
    out = _combine(res.results)
    if res.exec_time_ns is not None:
        print(f"HW exec time: {res.exec_time_ns} ns")
    return out
